# revision 59
# baseline (speedup 1.0000x reference)
"""Trainium2 Bass kernel for nn_CG_MSA_M (cross-gated multi-head channel attention).

Sharding: data-parallel over batch (8 samples -> 8 cores), weights replicated.

Per-core algorithm (one sample, C=96 channels, N=128x128 positions):
  - x,y are staged into zero-padded canvases twice: fp8(e4m3) canvases with a
    16-aligned row stride (144) for the conv taps, and fp16 canvases (stride
    130) for the dy=2 tap row + the 1x1 pos conv.
  - The depthwise 3x3 convs are folded into the producing 1x1 convs on the
    TensorEngine.  The dy=0/dy=1 tap rows run as fp8 DoubleRow matmuls (two
    taps contracted per instruction at 2 MACs/cell/cycle); the dy=2 taps run
    as fp16 matmuls against the fp16 canvas (precision).
  - The fuse 3x3 conv (192-ch contraction) pairs its v0/v_ halves per tap:
    9 DoubleRow matmuls per chunk instead of 18 fp16 ones.
  - Channel-attention Gram matrices accumulate via DMA transposes + PE
    matmuls in fp16; softmax path is fp16/fp32 (unchanged).
  - Output chunk = [W_proj@A | W_posX | W_posY] @ [v; x; y] in fp16,
    staged PSUM -> SBUF -> DRAM per chunk on alternating DMA queues.
  - PE keep-warm dummy matmuls bridge the phase-D serial softmax so the
    HAM clock gate stays at 8/8 into the output phase.
Overall quantization error (vs f32 reference) ~1.2e-2, inside the 2e-2 gate.
"""

import numpy as np
import ml_dtypes

import concourse.bass as bass
import concourse.tile as tile
from concourse import bacc, mybir
from concourse.ap import AP
from concourse.bass_utils import run_bass_kernel_spmd

F16 = mybir.dt.float16
F32 = mybir.dt.float32
F8 = mybir.dt.float8e4
E4NP = ml_dtypes.float8_e4m3

B, C, H, W, HEADS = 8, 96, 128, 128, 6
CH = C // HEADS
N = H * W  # 16384
WC16 = W + 2          # fp16 canvas row stride 130
CANV16 = (H + 2) * WC16
WC8 = 144             # fp8 canvas row stride (16-aligned for DoubleRow pairs)
CANV8 = (H + 2) * WC8
NCHUNK = 512
RPC = NCHUNK // W     # rows per chunk = 4
NCB = N // NCHUNK     # 32 chunks
DR = mybir.MatmulPerfMode.DoubleRow

TRACE = False
LAST_RESULTS = None


def _f8(a):
    return np.ascontiguousarray(np.asarray(a, np.float32).astype(E4NP))


def _f16(a):
    return np.ascontiguousarray(np.asarray(a, np.float32).astype(np.float16))


def _f32(a):
    return np.ascontiguousarray(np.asarray(a, np.float32))


def _prep_weights(w_pos, w_qv, w_qv_dw, w_kv, w_kv_dw, w_proj, w_fuse, b_fuse,
                  temperature):
    """Host-side weight composition (numpy)."""
    w_pos = w_pos[:, :, 0, 0]      # [192,192]
    w_qv = w_qv[:, :, 0, 0]        # [192,96]
    w_kv = w_kv[:, :, 0, 0]        # [192,96]
    w_proj = w_proj[:, :, 0, 0]    # [192,96]
    dwq = w_qv_dw[:, 0].reshape(2 * C, 9)   # [192,9]
    dwk = w_kv_dw[:, 0].reshape(2 * C, 9)   # [192,9]

    # wx9[i, t, c] = w_qv[c, i] * dwq[c, t]   (lhsT layout: [K=in, tap, M=out])
    wx9 = np.einsum('ci,ct->itc', w_qv, dwq)             # [96, 9, 192]
    wy9 = np.einsum('ci,ct->itc', w_kv, dwk)             # [96, 9, 192]

    def split_taps(w9, mlo, mhi):
        # pairs [96, 3(dx), 2(dy=0,1), M] fp8 ; singles [96, 3(dx), M] fp16
        pair = np.empty((C, 3, 2, mhi - mlo), np.float32)
        sing = np.empty((C, 3, mhi - mlo), np.float32)
        for dx in range(3):
            for j in range(2):
                pair[:, dx, j, :] = w9[:, j * 3 + dx, mlo:mhi]
            sing[:, dx, :] = w9[:, 6 + dx, mlo:mhi]
        return _f8(pair), _f16(sing)

    wq8, wqs = split_taps(wx9, 0, C)
    wv_8, wv_s = split_taps(wx9, C, 2 * C)
    wk8, wks = split_taps(wy9, 0, C)
    wv08, wv0s = split_taps(wy9, C, 2 * C)

    # fuse: wf8[k, t, half, m] = w_fuse[m, half*96 + k, t]
    wfr = w_fuse.reshape(C, 2 * C, 9)                    # [96 o, 192 j, 9 t]
    wf8 = np.empty((C, 9, 2, C), np.float32)
    for h in range(2):
        wf8[:, :, h, :] = np.transpose(wfr[:, h * C:(h + 1) * C, :], (1, 2, 0))

    temp_row = np.repeat(temperature.reshape(HEADS), CH).reshape(C, 1)

    # pack into 3 tensors (one DMA each; the gpsimd SWDGE queue pays
    # ~0.7us of descriptor-gen per dma_start, so fewer is faster)
    pk8 = np.concatenate(
        [wq8.reshape(C, -1), wk8.reshape(C, -1), wv_8.reshape(C, -1),
         wv08.reshape(C, -1), _f8(wf8).reshape(C, -1)], axis=1)
    pk16 = np.concatenate(
        [wqs.reshape(C, -1), wks.reshape(C, -1), wv_s.reshape(C, -1),
         wv0s.reshape(C, -1), _f16(w_proj.T), _f16(w_pos[:, :C].T),
         _f16(w_pos[:, C:].T)], axis=1)
    pk32 = np.concatenate(
        [_f32(temp_row), _f32(b_fuse.reshape(C, 1)), _f32(np.eye(C)),
         _f32(np.kron(np.eye(HEADS), np.ones((CH, CH))) * 30000.0
              - 30000.0)], axis=1)
    return {"pk8": np.ascontiguousarray(pk8),
            "pk16": np.ascontiguousarray(pk16),
            "pk32": np.ascontiguousarray(pk32)}


def _canvas16_view(canvas_ap, cb, dy, dx, rows=RPC):
    """[96, rows, 128] fp16-canvas view for chunk cb at shift (dy,dx)."""
    r = canvas_ap.rearrange("p (r c) -> p r c", c=WC16)
    return r[:, cb * RPC + dy:cb * RPC + dy + rows, dx:dx + W]


def _pair_view(base_ap, pstride, off, pair_stride):
    """[96, 2, RPC, 128] DoubleRow rhs view: two tap windows per partition."""
    return AP(tensor=base_ap.tensor, offset=base_ap.offset + off,
              ap=[[pstride, C], [pair_stride, 2], [WC8, RPC], [1, W]])


def _dummy_out(tc, nc, out_d):
    with tc.tile_pool(name="dummy", bufs=2) as dp:
        for mt, (o0, osz) in enumerate(((0, 128), (128, 64))):
            for cb in range(NCB):
                t = dp.tile([osz, NCHUNK], F32, tag=f"d{mt}")
                nc.vector.memset(t[:], 0.0)
                nc.sync.dma_start(
                    out=out_d[o0:o0 + osz, cb * NCHUNK:(cb + 1) * NCHUNK],
                    in_=t[:])


def _build_nc():
    nc = bacc.Bacc(None, name="cg_msa")

    x_d = nc.dram_tensor("x", [C, N], F32, kind="ExternalInput")
    y_d = nc.dram_tensor("y", [C, N], F32, kind="ExternalInput")
    w_d = {}
    wspec = [("pk8", [C, 4 * 576 + 1728], F8),
             ("pk16", [C, 4 * 288 + 3 * 192], F16),
             ("pk32", [C, 194], F32)]
    for nm, shp, dt in wspec:
        w_d[nm] = nc.dram_tensor(nm, shp, dt, kind="ExternalInput")
    out_d = nc.dram_tensor("out", [2 * C, N], F32, kind="ExternalOutput")

    with tile.TileContext(nc) as tc:
        _emit(tc, nc, x_d, y_d, w_d, out_d)
    nc.finalize()
    return nc


def _emit(tc, nc, x_d, y_d, w_d, out_d):
    import os
    from contextlib import ExitStack
    PH = os.environ.get("K_PHASES", "BCDEF")
    ctx = ExitStack()
    with ctx:
        const = ctx.enter_context(tc.tile_pool(name="const", bufs=1))
        canv = ctx.enter_context(tc.tile_pool(name="canv", bufs=1))
        big = ctx.enter_context(tc.tile_pool(name="big", bufs=1))

        # ---- canvases ----
        xc8 = canv.tile([C, CANV8], F8)
        yc8 = canv.tile([C, CANV8], F8)
        vpair = canv.tile([C, 2, CANV8], F8)   # j=0: v0 (from y), j=1: v_ (x)
        xc16 = canv.tile([C, CANV16], F16)
        yc16 = canv.tile([C, CANV16], F16)

        # PE warmup as early as possible (HAM clock-gate ramp): source tile
        # is memset on gpsimd, matmuls run while the first loads stream in.
        with tc.tile_pool(name="warm", bufs=1) as warm, \
             tc.tile_pool(name="warmps", bufs=1, space="PSUM") as warmps:
            wsc = warm.tile([C, NCHUNK], F16)
            nc.gpsimd.memset(wsc[:], 0.0)
            wps = warmps.tile([C, NCHUNK], F32)
            for _ in range(6):
                nc.tensor.matmul(wps[:], wsc[:, 0:C], wsc[:],
                                 start=True, stop=True)

        # ---- phase B: first input block loads (emitted before weights so the
        # sync/scalar DMA queues start streaming x,y immediately) ----
        ROWS_PER_LOAD = 4
        NLOAD = H // ROWS_PER_LOAD
        BCOLS = ROWS_PER_LOAD * W
        instg = ctx.enter_context(tc.tile_pool(name="instg", bufs=4))

        def emit_block(blk, fast=False):
            """Load one 4-row block of x,y and cast into the 4 canvases."""
            r0 = blk * ROWS_PER_LOAD
            stgs = []
            for si, src_d in enumerate((x_d, y_d)):
                stg = instg.tile([C, BCOLS], F32, tag=f"in{si}")
                # gpsimd SWDGE queue starts ~5us before sync/scalar HWDGE:
                # first blocks go there so chunk 0 can start early
                eng = nc.gpsimd if fast else (nc.sync if si == 0
                                              else nc.scalar)
                eng.dma_start(out=stg[:], in_=src_d[:, r0 * W:r0 * W + BCOLS])
                stgs.append(stg)
            src2d = [s[:].rearrange("p (r c) -> p r c", c=W) for s in stgs]
            # fp8 canvases (phase-C critical): both on vector (gpsimd casts
            # are 4x slower and contend with DVE)
            for si, cv8 in enumerate((xc8, yc8)):
                dst = cv8[:].rearrange("p (r c) -> p r c", c=WC8)[
                    :, r0 + 1:r0 + 1 + ROWS_PER_LOAD, 1:1 + W]
                nc.vector.tensor_copy(dst, src2d[si])
            # fp16 canvases (dy=2 taps + pos phase): x on scalar, y on vector
            for si, (cv16, ceng) in enumerate(((xc16, nc.scalar),
                                               (yc16, nc.vector))):
                dst = cv16[:].rearrange("p (r c) -> p r c", c=WC16)[
                    :, r0 + 1:r0 + 1 + ROWS_PER_LOAD, 1:1 + W]
                if ceng is nc.scalar:
                    ceng.copy(out=dst, in_=src2d[si])
                else:
                    ceng.tensor_copy(dst, src2d[si])

        emit_block(0, fast=True)
        emit_block(1, fast=True)

        # ---- weights to SBUF: 3 packed DMAs on the gpsimd queue, then
        # sliced APs per logical weight ----
        pkt = {}
        for nm in w_d:
            t = const.tile(list(w_d[nm].shape), w_d[nm].dtype, tag=f"w_{nm}")
            nc.gpsimd.dma_start(out=t[:], in_=w_d[nm][:])
            pkt[nm] = t
        p8, p16, p32 = pkt["pk8"][:], pkt["pk16"][:], pkt["pk32"][:]
        r4 = lambda ap: ap.rearrange("p (a b m) -> p a b m", b=2, m=C)
        r3 = lambda ap: ap.rearrange("p (a m) -> p a m", m=C)
        wsb = {
            "wq8": r4(p8[:, 0:576]), "wk8": r4(p8[:, 576:1152]),
            "wv_8": r4(p8[:, 1152:1728]), "wv08": r4(p8[:, 1728:2304]),
            "wf8": p8[:, 2304:4032].rearrange("p (t h m) -> p t h m",
                                              h=2, m=C),
            "wqs": r3(p16[:, 0:288]), "wks": r3(p16[:, 288:576]),
            "wv_s": r3(p16[:, 576:864]), "wv0s": r3(p16[:, 864:1152]),
            "wprojT": p16[:, 1152:1344], "wposxT": p16[:, 1344:1536],
            "wposyT": p16[:, 1536:1728],
            "temp_row": p32[:, 0:1], "bfuse": p32[:, 1:2],
            "identf": p32[:, 2:98], "mask": p32[:, 98:194],
        }

        # border zeros: one top+bottom memset and one L/R-column memset per
        # canvas (combined multi-dim APs)
        def borders(base, stride, ncols, eng):
            pstr = base.ap[0][0]
            tb = AP(tensor=base.tensor, offset=base.offset,
                    ap=[[pstr, C], [(H + 1) * stride, 2], [1, ncols]])
            eng.memset(tb, 0.0)
            lr = AP(tensor=base.tensor, offset=base.offset + stride,
                    ap=[[pstr, C], [stride, H], [W + 1, 2]])
            eng.memset(lr, 0.0)
        borders(xc8[:], WC8, W + 4, nc.gpsimd)
        borders(yc8[:], WC8, W + 4, nc.gpsimd)
        borders(vpair[:, 0, :], WC8, W + 4, nc.gpsimd)
        borders(vpair[:, 1, :], WC8, W + 4, nc.gpsimd)
        borders(xc16[:], WC16, W + 2, nc.vector)
        borders(yc16[:], WC16, W + 2, nc.vector)

        LOOK = 4  # blocks of emission lookahead into the chunk loop
        for blk in range(2, LOOK):
            emit_block(blk)

        v_sb = big.tile([C, N], F16)
        stats = ctx.enter_context(tc.tile_pool(name="stats", bufs=1))
        nqp = stats.tile([C, NCB], F32, tag="nqp")
        nkp = stats.tile([C, NCB], F32, tag="nkp")
        if "B" not in PH:
            _dummy_out(tc, nc, out_d)
            return

        # ---- phase C: q,k,v_,v0 + gram accumulation + fuse ----
        if "C" not in PH:
            _dummy_out(tc, nc, out_d)
            return
        from contextlib import ExitStack as _ES
        gctx = _ES()
        gpool = gctx.enter_context(tc.tile_pool(name="gps", bufs=1, space="PSUM"))
        g_ps = gpool.tile([C, C], F32)  # Gqk

        xc8b, yc8b, vpb = xc8[:], yc8[:], vpair[:]
        ps8x = xc8b.ap[0][0]
        ps8v = vpb.ap[0][0]

        def conv_group(ps, w8, ws16, cv8b, pstr, cv16, cb):
            # 3 DoubleRow pairs (dy=0,1) + 3 fp16 singles (dy=2)
            for dx in range(3):
                rhs = _pair_view(cv8b, pstr, (cb * RPC) * WC8 + dx, WC8)
                nc.tensor.matmul(ps[:], w8[:, dx, :, :], rhs,
                                 start=(dx == 0), stop=False, perf_mode=DR)
            for dx in range(3):
                nc.tensor.matmul(ps[:], ws16[:, dx, :],
                                 _canvas16_view(cv16[:], cb, 2, dx),
                                 start=False, stop=(dx == 2))

        def fuse_chunk(psE, cb):
            ps = psE.tile([C, NCHUNK], F32, tag="v")
            for t in range(9):
                dy, dx = t // 3, t % 3
                rhs = _pair_view(vpb, ps8v, (cb * RPC + dy) * WC8 + dx, CANV8)
                nc.tensor.matmul(ps[:], wsb["wf8"][:, t, :, :], rhs,
                                 start=(t == 0), stop=(t == 8), perf_mode=DR)
            nc.scalar.activation(
                v_sb[:, cb * NCHUNK:(cb + 1) * NCHUNK], ps[:],
                mybir.ActivationFunctionType.Identity,
                bias=wsb["bfuse"][:], scale=1.0)

        with tc.tile_pool(name="psC", bufs=6, space="PSUM") as psC, \
             tc.tile_pool(name="psE", bufs=1, space="PSUM") as psE, \
             tc.tile_pool(name="stC", bufs=3) as stC, \
             tc.tile_pool(name="sqp", bufs=1) as sqp, \
             tc.tile_pool(name="stT", bufs=3) as stT:
            def gram_chunk(tps, cb):
                for j in range(RPC):
                    st = (cb == 0 and j == 0)
                    sp = (cb == NCB - 1 and j == RPC - 1)
                    nc.tensor.matmul(
                        g_ps[:], tps[:, 0, j, :], tps[:, 1, j, :],
                        start=st, stop=sp, skip_group_check=True)

            tps_prev = None
            for cb in range(NCB):
                if cb + LOOK < NLOAD:
                    emit_block(cb + LOOK)
                sb2 = stC.tile([C, 2, NCHUNK], F16, tag="qk")
                for side in range(2):
                    cv8b, pstr, cv16 = ((xc8b, ps8x, xc16) if side == 0
                                        else (yc8b, ps8x, yc16))
                    wp8 = wsb["wq8"] if side == 0 else wsb["wk8"]
                    wps16 = wsb["wqs"] if side == 0 else wsb["wks"]
                    # q / k tile
                    ps = psC.tile([C, NCHUNK], F32, tag="qv")
                    conv_group(ps, wp8, wps16, cv8b, pstr, cv16, cb)
                    sb = sb2[:, side, :]
                    nc.vector.tensor_copy(sb, ps[:])
                    sq = sqp.tile([C, NCHUNK], F16, tag="sq")
                    npart = nqp if side == 0 else nkp
                    nc.scalar.activation(
                        sq[:], sb, mybir.ActivationFunctionType.Square,
                        accum_out=npart[:, cb:cb + 1])
                    # v_ / v0 tile
                    wv8 = wsb["wv_8"] if side == 0 else wsb["wv08"]
                    wvs16 = wsb["wv_s"] if side == 0 else wsb["wv0s"]
                    ps2 = psC.tile([C, NCHUNK], F32, tag="qv")
                    conv_group(ps2, wv8, wvs16, cv8b, pstr, cv16, cb)
                    j = 1 if side == 0 else 0
                    dst = vpair[:, j, :].rearrange("p (r c) -> p r c", c=WC8)[
                        :, cb * RPC + 1:cb * RPC + 1 + RPC, 1:1 + W]
                    nc.vector.tensor_copy(
                        dst, ps2[:].rearrange("p (r c) -> p r c", c=W))

                tps = stT.tile([W, 2, RPC, C], F16)
                nc.sync.dma_start_transpose(tps[:], sb2[:])
                if cb >= 1:
                    gram_chunk(tps_prev, cb - 1)
                    fuse_chunk(psE, cb - 1)
                tps_prev = tps
            gram_chunk(tps_prev, NCB - 1)
            fuse_chunk(psE, NCB - 1)

        # ---- phase D: norms, softmax, M1T ----
        if "D" not in PH:
            gctx.close()
            _dummy_out(tc, nc, out_d)
            return
        smx = ctx.enter_context(tc.tile_pool(name="smx", bufs=1))
        with tc.tile_pool(name="psD", bufs=1, space="PSUM") as psD:
            # keep-warm: PE dummy matmuls interleaved with phase D's serial
            # chain so the HAM clock gate stays at 8/8 into phase F.
            wsc2 = smx.tile([C, NCHUNK], F16, tag="warmsrc")
            nc.gpsimd.memset(wsc2[:], 0.0)
            wps2 = psD.tile([C, NCHUNK], F32, tag="warm")

            def keep_warm(n):
                for _ in range(n):
                    nc.tensor.matmul(wps2[:], wsc2[:, 0:C], wsc2[:],
                                     start=True, stop=True)

            g_sb = smx.tile([C, C], F32)
            nc.vector.tensor_copy(g_sb[:], g_ps[:])
            keep_warm(4)

            rr = {}
            for npart, nm in ((nqp, "q"), (nkp, "k")):
                nrm2 = smx.tile([C, 1], F32, tag=f"n{nm}")
                nc.vector.tensor_reduce(
                    nrm2[:], npart[:], axis=mybir.AxisListType.X,
                    op=mybir.AluOpType.add)
                nrm = smx.tile([C, 1], F32, tag=f"s{nm}")
                nc.scalar.sqrt(nrm[:], nrm2[:])
                nc.vector.tensor_scalar_max(nrm[:], nrm[:], 1e-12)
                rinv = smx.tile([C, 1], F32, tag=f"r{nm}")
                nc.vector.reciprocal(rinv[:], nrm[:])
                rr[nm] = rinv
            nc.vector.tensor_tensor(
                rr["q"][:], rr["q"][:], wsb["temp_row"][:],
                mybir.AluOpType.mult)

            rows = {}
            for nm in ("q", "k"):
                rp = psD.tile([1, C], F32, tag="row")
                nc.tensor.transpose(rp[:], rr[nm][:], wsb["identf"][:])
                rs = smx.tile([1, C], F32, tag=f"row{nm}")
                nc.vector.tensor_copy(rs[:], rp[:])
                rows[nm] = rs
                keep_warm(3)
            r_ps = psD.tile([C, C], F32, tag="R")
            nc.tensor.matmul(r_ps[:], rows["q"][:], rows["k"][:])
            keep_warm(4)
            logits = smx.tile([C, C], F32)
            nc.vector.tensor_tensor(
                logits[:], g_sb[:], r_ps[:], mybir.AluOpType.mult)
            nc.vector.tensor_tensor(
                logits[:], logits[:], wsb["mask"][:], mybir.AluOpType.add)

            mx = smx.tile([C, 1], F32)
            nc.vector.tensor_reduce(
                mx[:], logits[:], axis=mybir.AxisListType.X,
                op=mybir.AluOpType.max, negate=True)
            e = smx.tile([C, C], F32)
            nc.scalar.activation(
                e[:], logits[:], mybir.ActivationFunctionType.Exp,
                bias=mx[:], scale=1.0)
            s = smx.tile([C, 1], F32)
            nc.vector.tensor_reduce(
                s[:], e[:], axis=mybir.AxisListType.X, op=mybir.AluOpType.add)
            rs = smx.tile([C, 1], F32)
            nc.vector.reciprocal(rs[:], s[:])
            a_sb = smx.tile([C, C], F16)
            nc.scalar.mul(a_sb[:], e[:], rs[:])

            keep_warm(6)
            m1_ps = psD.tile([C, 2 * C], F32, tag="m1")
            nc.tensor.matmul(m1_ps[:], a_sb[:], wsb["wprojT"][:])
            m1T = smx.tile([C, 2 * C], F16)
            nc.vector.tensor_copy(m1T[:], m1_ps[:])
        gctx.close()

        # ---- phase F: out = M1 @ v + W_pos @ [x;y], PSUM -> DRAM direct ----
        if "F" not in PH:
            _dummy_out(tc, nc, out_d)
            return
        with tc.tile_pool(name="psF", bufs=4, space="PSUM") as psF, \
             tc.tile_pool(name="ostg", bufs=2) as ostg:
            for cb in range(NCB):
                for mt, (o0, osz) in enumerate(((0, 128), (128, 64))):
                    ps = psF.tile([osz, NCHUNK], F32, tag=f"o{mt}")
                    nc.tensor.matmul(
                        ps[:], wsb["wposxT"][:, o0:o0 + osz],
                        _canvas16_view(xc16[:], cb, 1, 1),
                        start=True, stop=False)
                    nc.tensor.matmul(
                        ps[:], wsb["wposyT"][:, o0:o0 + osz],
                        _canvas16_view(yc16[:], cb, 1, 1),
                        start=False, stop=False)
                    nc.tensor.matmul(
                        ps[:], m1T[:, o0:o0 + osz],
                        v_sb[:, cb * NCHUNK:(cb + 1) * NCHUNK],
                        start=False, stop=True)
                    osb = ostg.tile([osz, NCHUNK], F32, tag=f"os{mt}")
                    # halve PSUM-free latency: split every copy across both
                    # engines (keeps the PE out of mid-pstate micro-gaps)
                    h = NCHUNK // 2
                    nc.scalar.copy(out=osb[:, 0:h], in_=ps[:, 0:h])
                    nc.vector.tensor_copy(osb[:, h:], ps[:, h:])
                    oeng = nc.sync if (cb + mt) % 2 == 0 else nc.scalar
                    oeng.dma_start(
                        out=out_d[o0:o0 + osz,
                                  cb * NCHUNK:(cb + 1) * NCHUNK],
                        in_=osb[:])


_NC_CACHE = None


def kernel(x, y, w_pos, w_qv, w_qv_dw, w_kv, w_kv_dw, w_proj, w_fuse, b_fuse,
           temperature):
    global _NC_CACHE, LAST_RESULTS
    x = _f32(np.asarray(x))
    y = _f32(np.asarray(y))
    wts = _prep_weights(
        np.asarray(w_pos, np.float32), np.asarray(w_qv, np.float32),
        np.asarray(w_qv_dw, np.float32), np.asarray(w_kv, np.float32),
        np.asarray(w_kv_dw, np.float32), np.asarray(w_proj, np.float32),
        np.asarray(w_fuse, np.float32), np.asarray(b_fuse, np.float32),
        np.asarray(temperature, np.float32))

    if _NC_CACHE is None:
        _NC_CACHE = _build_nc()
    nc = _NC_CACHE

    in_maps = []
    for core in range(B):
        m = {"x": np.ascontiguousarray(x[core].reshape(C, N)),
             "y": np.ascontiguousarray(y[core].reshape(C, N))}
        m.update(wts)
        in_maps.append(m)

    res = run_bass_kernel_spmd(nc, in_maps, core_ids=list(range(B)),
                               trace=TRACE)
    LAST_RESULTS = res
    out = np.stack([np.asarray(r["out"]) for r in res.results])
    return out.reshape(B, 2 * C, H, W).astype(np.float32)


if __name__ == "__main__":
    print("built nc ok" if _build_nc() else "")


# revision 60
# speedup vs baseline: 1.0060x; 1.0060x over previous
"""Trainium2 Bass kernel for nn_CG_MSA_M (cross-gated multi-head channel attention).

Sharding: data-parallel over batch (8 samples -> 8 cores), weights replicated.

Per-core algorithm (one sample, C=96 channels, N=128x128 positions):
  - x,y are staged into zero-padded canvases twice: fp8(e4m3) canvases with a
    16-aligned row stride (144) for the conv taps, and fp16 canvases (stride
    130) for the dy=2 tap row + the 1x1 pos conv.
  - The depthwise 3x3 convs are folded into the producing 1x1 convs on the
    TensorEngine.  The dy=0/dy=1 tap rows run as fp8 DoubleRow matmuls (two
    taps contracted per instruction at 2 MACs/cell/cycle); the dy=2 taps run
    as fp16 matmuls against the fp16 canvas (precision).
  - The fuse 3x3 conv (192-ch contraction) pairs its v0/v_ halves per tap:
    9 DoubleRow matmuls per chunk instead of 18 fp16 ones.
  - Channel-attention Gram matrices accumulate via DMA transposes + PE
    matmuls in fp16; softmax path is fp16/fp32 (unchanged).
  - Output chunk = [W_proj@A | W_posX | W_posY] @ [v; x; y] in fp16,
    staged PSUM -> SBUF -> DRAM per chunk on alternating DMA queues.
  - PE keep-warm dummy matmuls bridge the phase-D serial softmax so the
    HAM clock gate stays at 8/8 into the output phase.
Overall quantization error (vs f32 reference) ~1.2e-2, inside the 2e-2 gate.
"""

import numpy as np
import ml_dtypes

import concourse.bass as bass
import concourse.tile as tile
from concourse import bacc, mybir
from concourse.ap import AP
from concourse.bass_utils import run_bass_kernel_spmd

F16 = mybir.dt.float16
F32 = mybir.dt.float32
F8 = mybir.dt.float8e4
E4NP = ml_dtypes.float8_e4m3

B, C, H, W, HEADS = 8, 96, 128, 128, 6
CH = C // HEADS
N = H * W  # 16384
WC16 = W + 2          # fp16 canvas row stride 130
CANV16 = (H + 2) * WC16
WC8 = 144             # fp8 canvas row stride (16-aligned for DoubleRow pairs)
CANV8 = (H + 2) * WC8
NCHUNK = 512
RPC = NCHUNK // W     # rows per chunk = 4
NCB = N // NCHUNK     # 32 chunks
DR = mybir.MatmulPerfMode.DoubleRow

TRACE = False
LAST_RESULTS = None


def _f8(a):
    return np.ascontiguousarray(np.asarray(a, np.float32).astype(E4NP))


def _f16(a):
    return np.ascontiguousarray(np.asarray(a, np.float32).astype(np.float16))


def _f32(a):
    return np.ascontiguousarray(np.asarray(a, np.float32))


def _prep_weights(w_pos, w_qv, w_qv_dw, w_kv, w_kv_dw, w_proj, w_fuse, b_fuse,
                  temperature):
    """Host-side weight composition (numpy)."""
    w_pos = w_pos[:, :, 0, 0]      # [192,192]
    w_qv = w_qv[:, :, 0, 0]        # [192,96]
    w_kv = w_kv[:, :, 0, 0]        # [192,96]
    w_proj = w_proj[:, :, 0, 0]    # [192,96]
    dwq = w_qv_dw[:, 0].reshape(2 * C, 9)   # [192,9]
    dwk = w_kv_dw[:, 0].reshape(2 * C, 9)   # [192,9]

    # wx9[i, t, c] = w_qv[c, i] * dwq[c, t]   (lhsT layout: [K=in, tap, M=out])
    wx9 = np.einsum('ci,ct->itc', w_qv, dwq)             # [96, 9, 192]
    wy9 = np.einsum('ci,ct->itc', w_kv, dwk)             # [96, 9, 192]

    def split_taps(w9, mlo, mhi):
        # pairs [96, 3(dx), 2(dy=0,1), M] fp8 ; singles [96, 3(dx), M] fp16
        pair = np.empty((C, 3, 2, mhi - mlo), np.float32)
        sing = np.empty((C, 3, mhi - mlo), np.float32)
        for dx in range(3):
            for j in range(2):
                pair[:, dx, j, :] = w9[:, j * 3 + dx, mlo:mhi]
            sing[:, dx, :] = w9[:, 6 + dx, mlo:mhi]
        return _f8(pair), _f16(sing)

    wq8, wqs = split_taps(wx9, 0, C)
    wv_8, wv_s = split_taps(wx9, C, 2 * C)
    wk8, wks = split_taps(wy9, 0, C)
    wv08, wv0s = split_taps(wy9, C, 2 * C)

    # fuse: wf8[k, t, half, m] = w_fuse[m, half*96 + k, t]
    wfr = w_fuse.reshape(C, 2 * C, 9)                    # [96 o, 192 j, 9 t]
    wf8 = np.empty((C, 9, 2, C), np.float32)
    for h in range(2):
        wf8[:, :, h, :] = np.transpose(wfr[:, h * C:(h + 1) * C, :], (1, 2, 0))

    temp_row = np.repeat(temperature.reshape(HEADS), CH).reshape(C, 1)

    # pack into 3 tensors (one DMA each; the gpsimd SWDGE queue pays
    # ~0.7us of descriptor-gen per dma_start, so fewer is faster)
    pk8 = np.concatenate(
        [wq8.reshape(C, -1), wk8.reshape(C, -1), wv_8.reshape(C, -1),
         wv08.reshape(C, -1), _f8(wf8).reshape(C, -1)], axis=1)
    pk16 = np.concatenate(
        [wqs.reshape(C, -1), wks.reshape(C, -1), wv_s.reshape(C, -1),
         wv0s.reshape(C, -1), _f16(w_proj.T), _f16(w_pos[:, :C].T),
         _f16(w_pos[:, C:].T)], axis=1)
    pk32 = np.concatenate(
        [_f32(temp_row), _f32(b_fuse.reshape(C, 1)), _f32(np.eye(C)),
         _f32(np.kron(np.eye(HEADS), np.ones((CH, CH))) * 30000.0
              - 30000.0)], axis=1)
    return {"pk8": np.ascontiguousarray(pk8),
            "pk16": np.ascontiguousarray(pk16),
            "pk32": np.ascontiguousarray(pk32)}


def _canvas16_view(canvas_ap, cb, dy, dx, rows=RPC):
    """[96, rows, 128] fp16-canvas view for chunk cb at shift (dy,dx)."""
    r = canvas_ap.rearrange("p (r c) -> p r c", c=WC16)
    return r[:, cb * RPC + dy:cb * RPC + dy + rows, dx:dx + W]


def _pair_view(base_ap, pstride, off, pair_stride):
    """[96, 2, RPC, 128] DoubleRow rhs view: two tap windows per partition."""
    return AP(tensor=base_ap.tensor, offset=base_ap.offset + off,
              ap=[[pstride, C], [pair_stride, 2], [WC8, RPC], [1, W]])


def _dummy_out(tc, nc, out_d):
    with tc.tile_pool(name="dummy", bufs=2) as dp:
        for mt, (o0, osz) in enumerate(((0, 128), (128, 64))):
            for cb in range(NCB):
                t = dp.tile([osz, NCHUNK], F32, tag=f"d{mt}")
                nc.vector.memset(t[:], 0.0)
                nc.sync.dma_start(
                    out=out_d[o0:o0 + osz, cb * NCHUNK:(cb + 1) * NCHUNK],
                    in_=t[:])


def _build_nc():
    nc = bacc.Bacc(None, name="cg_msa")

    x_d = nc.dram_tensor("x", [C, N], F32, kind="ExternalInput")
    y_d = nc.dram_tensor("y", [C, N], F32, kind="ExternalInput")
    w_d = {}
    wspec = [("pk8", [C, 4 * 576 + 1728], F8),
             ("pk16", [C, 4 * 288 + 3 * 192], F16),
             ("pk32", [C, 194], F32)]
    for nm, shp, dt in wspec:
        w_d[nm] = nc.dram_tensor(nm, shp, dt, kind="ExternalInput")
    out_d = nc.dram_tensor("out", [2 * C, N], F32, kind="ExternalOutput")

    with tile.TileContext(nc) as tc:
        _emit(tc, nc, x_d, y_d, w_d, out_d)
    nc.finalize()
    return nc


def _emit(tc, nc, x_d, y_d, w_d, out_d):
    import os
    from contextlib import ExitStack
    PH = os.environ.get("K_PHASES", "BCDEF")
    ctx = ExitStack()
    with ctx:
        const = ctx.enter_context(tc.tile_pool(name="const", bufs=1))
        canv = ctx.enter_context(tc.tile_pool(name="canv", bufs=1))
        big = ctx.enter_context(tc.tile_pool(name="big", bufs=1))

        # ---- canvases ----
        xc8 = canv.tile([C, CANV8], F8)
        yc8 = canv.tile([C, CANV8], F8)
        vpair = canv.tile([C, 2, CANV8], F8)   # j=0: v0 (from y), j=1: v_ (x)
        xc16 = canv.tile([C, CANV16], F16)
        yc16 = canv.tile([C, CANV16], F16)

        # PE warmup as early as possible (HAM clock-gate ramp): source tile
        # is memset on gpsimd, matmuls run while the first loads stream in.
        with tc.tile_pool(name="warm", bufs=1) as warm, \
             tc.tile_pool(name="warmps", bufs=1, space="PSUM") as warmps:
            wsc = warm.tile([C, NCHUNK], F16)
            nc.gpsimd.memset(wsc[:], 0.0)
            wps = warmps.tile([C, NCHUNK], F32)
            for _ in range(10):
                nc.tensor.matmul(wps[:], wsc[:, 0:C], wsc[:],
                                 start=True, stop=True)

        # ---- phase B: first input block loads (emitted before weights so the
        # sync/scalar DMA queues start streaming x,y immediately) ----
        ROWS_PER_LOAD = 4
        NLOAD = H // ROWS_PER_LOAD
        BCOLS = ROWS_PER_LOAD * W
        instg = ctx.enter_context(tc.tile_pool(name="instg", bufs=4))

        def emit_block(blk):
            """Load one 4-row block of x,y and cast into the 4 canvases."""
            r0 = blk * ROWS_PER_LOAD
            stgs = []
            for si, src_d in enumerate((x_d, y_d)):
                stg = instg.tile([C, BCOLS], F32, tag=f"in{si}")
                eng = nc.sync if si == 0 else nc.scalar
                eng.dma_start(out=stg[:], in_=src_d[:, r0 * W:r0 * W + BCOLS])
                stgs.append(stg)
            src2d = [s[:].rearrange("p (r c) -> p r c", c=W) for s in stgs]
            # fp8 canvases (phase-C critical): both on vector (gpsimd casts
            # are 4x slower and contend with DVE)
            for si, cv8 in enumerate((xc8, yc8)):
                dst = cv8[:].rearrange("p (r c) -> p r c", c=WC8)[
                    :, r0 + 1:r0 + 1 + ROWS_PER_LOAD, 1:1 + W]
                nc.vector.tensor_copy(dst, src2d[si])
            # fp16 canvases (dy=2 taps + pos phase): x on scalar, y on vector
            for si, (cv16, ceng) in enumerate(((xc16, nc.scalar),
                                               (yc16, nc.vector))):
                dst = cv16[:].rearrange("p (r c) -> p r c", c=WC16)[
                    :, r0 + 1:r0 + 1 + ROWS_PER_LOAD, 1:1 + W]
                if ceng is nc.scalar:
                    ceng.copy(out=dst, in_=src2d[si])
                else:
                    ceng.tensor_copy(dst, src2d[si])

        # ---- weights to SBUF: 3 packed DMAs on the gpsimd queue, then
        # sliced APs per logical weight ----
        pkt = {}
        for nm in w_d:
            t = const.tile(list(w_d[nm].shape), w_d[nm].dtype, tag=f"w_{nm}")
            nc.gpsimd.dma_start(out=t[:], in_=w_d[nm][:])
            pkt[nm] = t
        p8, p16, p32 = pkt["pk8"][:], pkt["pk16"][:], pkt["pk32"][:]
        r4 = lambda ap: ap.rearrange("p (a b m) -> p a b m", b=2, m=C)
        r3 = lambda ap: ap.rearrange("p (a m) -> p a m", m=C)
        wsb = {
            "wq8": r4(p8[:, 0:576]), "wk8": r4(p8[:, 576:1152]),
            "wv_8": r4(p8[:, 1152:1728]), "wv08": r4(p8[:, 1728:2304]),
            "wf8": p8[:, 2304:4032].rearrange("p (t h m) -> p t h m",
                                              h=2, m=C),
            "wqs": r3(p16[:, 0:288]), "wks": r3(p16[:, 288:576]),
            "wv_s": r3(p16[:, 576:864]), "wv0s": r3(p16[:, 864:1152]),
            "wprojT": p16[:, 1152:1344], "wposxT": p16[:, 1344:1536],
            "wposyT": p16[:, 1536:1728],
            "temp_row": p32[:, 0:1], "bfuse": p32[:, 1:2],
            "identf": p32[:, 2:98], "mask": p32[:, 98:194],
        }

        # border zeros: one top+bottom memset and one L/R-column memset per
        # canvas (combined multi-dim APs)
        def borders(base, stride, ncols, eng):
            pstr = base.ap[0][0]
            tb = AP(tensor=base.tensor, offset=base.offset,
                    ap=[[pstr, C], [(H + 1) * stride, 2], [1, ncols]])
            eng.memset(tb, 0.0)
            lr = AP(tensor=base.tensor, offset=base.offset + stride,
                    ap=[[pstr, C], [stride, H], [W + 1, 2]])
            eng.memset(lr, 0.0)
        borders(xc8[:], WC8, W + 4, nc.gpsimd)
        borders(yc8[:], WC8, W + 4, nc.gpsimd)
        borders(vpair[:, 0, :], WC8, W + 4, nc.gpsimd)
        borders(vpair[:, 1, :], WC8, W + 4, nc.gpsimd)
        borders(xc16[:], WC16, W + 2, nc.vector)
        borders(yc16[:], WC16, W + 2, nc.vector)

        LOOK = 4  # blocks of emission lookahead into the chunk loop
        for blk in range(LOOK):
            emit_block(blk)

        v_sb = big.tile([C, N], F16)
        stats = ctx.enter_context(tc.tile_pool(name="stats", bufs=1))
        nqp = stats.tile([C, NCB], F32, tag="nqp")
        nkp = stats.tile([C, NCB], F32, tag="nkp")
        if "B" not in PH:
            _dummy_out(tc, nc, out_d)
            return

        # ---- phase C: q,k,v_,v0 + gram accumulation + fuse ----
        if "C" not in PH:
            _dummy_out(tc, nc, out_d)
            return
        from contextlib import ExitStack as _ES
        gctx = _ES()
        gpool = gctx.enter_context(tc.tile_pool(name="gps", bufs=1, space="PSUM"))
        g_ps = gpool.tile([C, C], F32)  # Gqk

        xc8b, yc8b, vpb = xc8[:], yc8[:], vpair[:]
        ps8x = xc8b.ap[0][0]
        ps8v = vpb.ap[0][0]

        def conv_group(ps, w8, ws16, cv8b, pstr, cv16, cb):
            # 3 DoubleRow pairs (dy=0,1) + 3 fp16 singles (dy=2)
            for dx in range(3):
                rhs = _pair_view(cv8b, pstr, (cb * RPC) * WC8 + dx, WC8)
                nc.tensor.matmul(ps[:], w8[:, dx, :, :], rhs,
                                 start=(dx == 0), stop=False, perf_mode=DR)
            for dx in range(3):
                nc.tensor.matmul(ps[:], ws16[:, dx, :],
                                 _canvas16_view(cv16[:], cb, 2, dx),
                                 start=False, stop=(dx == 2))

        def fuse_chunk(psE, cb):
            ps = psE.tile([C, NCHUNK], F32, tag="v")
            for t in range(9):
                dy, dx = t // 3, t % 3
                rhs = _pair_view(vpb, ps8v, (cb * RPC + dy) * WC8 + dx, CANV8)
                nc.tensor.matmul(ps[:], wsb["wf8"][:, t, :, :], rhs,
                                 start=(t == 0), stop=(t == 8), perf_mode=DR)
            nc.scalar.activation(
                v_sb[:, cb * NCHUNK:(cb + 1) * NCHUNK], ps[:],
                mybir.ActivationFunctionType.Identity,
                bias=wsb["bfuse"][:], scale=1.0)

        with tc.tile_pool(name="psC", bufs=6, space="PSUM") as psC, \
             tc.tile_pool(name="psE", bufs=1, space="PSUM") as psE, \
             tc.tile_pool(name="stC", bufs=3) as stC, \
             tc.tile_pool(name="sqp", bufs=1) as sqp, \
             tc.tile_pool(name="stT", bufs=3) as stT:
            def gram_chunk(tps, cb):
                for j in range(RPC):
                    st = (cb == 0 and j == 0)
                    sp = (cb == NCB - 1 and j == RPC - 1)
                    nc.tensor.matmul(
                        g_ps[:], tps[:, 0, j, :], tps[:, 1, j, :],
                        start=st, stop=sp, skip_group_check=True)

            tps_prev = None
            for cb in range(NCB):
                if cb + LOOK < NLOAD:
                    emit_block(cb + LOOK)
                sb2 = stC.tile([C, 2, NCHUNK], F16, tag="qk")
                for side in range(2):
                    cv8b, pstr, cv16 = ((xc8b, ps8x, xc16) if side == 0
                                        else (yc8b, ps8x, yc16))
                    wp8 = wsb["wq8"] if side == 0 else wsb["wk8"]
                    wps16 = wsb["wqs"] if side == 0 else wsb["wks"]
                    # q / k tile
                    ps = psC.tile([C, NCHUNK], F32, tag="qv")
                    conv_group(ps, wp8, wps16, cv8b, pstr, cv16, cb)
                    sb = sb2[:, side, :]
                    nc.vector.tensor_copy(sb, ps[:])
                    sq = sqp.tile([C, NCHUNK], F16, tag="sq")
                    npart = nqp if side == 0 else nkp
                    nc.scalar.activation(
                        sq[:], sb, mybir.ActivationFunctionType.Square,
                        accum_out=npart[:, cb:cb + 1])
                    # v_ / v0 tile
                    wv8 = wsb["wv_8"] if side == 0 else wsb["wv08"]
                    wvs16 = wsb["wv_s"] if side == 0 else wsb["wv0s"]
                    ps2 = psC.tile([C, NCHUNK], F32, tag="qv")
                    conv_group(ps2, wv8, wvs16, cv8b, pstr, cv16, cb)
                    j = 1 if side == 0 else 0
                    dst = vpair[:, j, :].rearrange("p (r c) -> p r c", c=WC8)[
                        :, cb * RPC + 1:cb * RPC + 1 + RPC, 1:1 + W]
                    nc.vector.tensor_copy(
                        dst, ps2[:].rearrange("p (r c) -> p r c", c=W))

                tps = stT.tile([W, 2, RPC, C], F16)
                nc.sync.dma_start_transpose(tps[:], sb2[:])
                if cb >= 1:
                    gram_chunk(tps_prev, cb - 1)
                    fuse_chunk(psE, cb - 1)
                tps_prev = tps
            gram_chunk(tps_prev, NCB - 1)
            fuse_chunk(psE, NCB - 1)

        # ---- phase D: norms, softmax, M1T ----
        if "D" not in PH:
            gctx.close()
            _dummy_out(tc, nc, out_d)
            return
        smx = ctx.enter_context(tc.tile_pool(name="smx", bufs=1))
        with tc.tile_pool(name="psD", bufs=1, space="PSUM") as psD:
            # keep-warm: PE dummy matmuls interleaved with phase D's serial
            # chain so the HAM clock gate stays at 8/8 into phase F.
            wsc2 = smx.tile([C, NCHUNK], F16, tag="warmsrc")
            nc.gpsimd.memset(wsc2[:], 0.0)
            wps2 = psD.tile([C, NCHUNK], F32, tag="warm")

            def keep_warm(n):
                for _ in range(n):
                    nc.tensor.matmul(wps2[:], wsc2[:, 0:C], wsc2[:],
                                     start=True, stop=True)

            g_sb = smx.tile([C, C], F32)
            nc.vector.tensor_copy(g_sb[:], g_ps[:])
            keep_warm(4)

            rr = {}
            for npart, nm in ((nqp, "q"), (nkp, "k")):
                nrm2 = smx.tile([C, 1], F32, tag=f"n{nm}")
                nc.vector.tensor_reduce(
                    nrm2[:], npart[:], axis=mybir.AxisListType.X,
                    op=mybir.AluOpType.add)
                nrm = smx.tile([C, 1], F32, tag=f"s{nm}")
                nc.scalar.sqrt(nrm[:], nrm2[:])
                nc.vector.tensor_scalar_max(nrm[:], nrm[:], 1e-12)
                rinv = smx.tile([C, 1], F32, tag=f"r{nm}")
                nc.vector.reciprocal(rinv[:], nrm[:])
                rr[nm] = rinv
            nc.vector.tensor_tensor(
                rr["q"][:], rr["q"][:], wsb["temp_row"][:],
                mybir.AluOpType.mult)

            rows = {}
            for nm in ("q", "k"):
                rp = psD.tile([1, C], F32, tag="row")
                nc.tensor.transpose(rp[:], rr[nm][:], wsb["identf"][:])
                rs = smx.tile([1, C], F32, tag=f"row{nm}")
                nc.vector.tensor_copy(rs[:], rp[:])
                rows[nm] = rs
                keep_warm(3)
            r_ps = psD.tile([C, C], F32, tag="R")
            nc.tensor.matmul(r_ps[:], rows["q"][:], rows["k"][:])
            keep_warm(4)
            logits = smx.tile([C, C], F32)
            nc.vector.tensor_tensor(
                logits[:], g_sb[:], r_ps[:], mybir.AluOpType.mult)
            nc.vector.tensor_tensor(
                logits[:], logits[:], wsb["mask"][:], mybir.AluOpType.add)

            mx = smx.tile([C, 1], F32)
            nc.vector.tensor_reduce(
                mx[:], logits[:], axis=mybir.AxisListType.X,
                op=mybir.AluOpType.max, negate=True)
            e = smx.tile([C, C], F32)
            nc.scalar.activation(
                e[:], logits[:], mybir.ActivationFunctionType.Exp,
                bias=mx[:], scale=1.0)
            s = smx.tile([C, 1], F32)
            nc.vector.tensor_reduce(
                s[:], e[:], axis=mybir.AxisListType.X, op=mybir.AluOpType.add)
            rs = smx.tile([C, 1], F32)
            nc.vector.reciprocal(rs[:], s[:])
            a_sb = smx.tile([C, C], F16)
            nc.scalar.mul(a_sb[:], e[:], rs[:])

            keep_warm(6)
            m1_ps = psD.tile([C, 2 * C], F32, tag="m1")
            nc.tensor.matmul(m1_ps[:], a_sb[:], wsb["wprojT"][:])
            m1T = smx.tile([C, 2 * C], F16)
            nc.vector.tensor_copy(m1T[:], m1_ps[:])
        gctx.close()

        # ---- phase F: out = M1 @ v + W_pos @ [x;y], PSUM -> DRAM direct ----
        if "F" not in PH:
            _dummy_out(tc, nc, out_d)
            return
        with tc.tile_pool(name="psF", bufs=4, space="PSUM") as psF, \
             tc.tile_pool(name="ostg", bufs=2) as ostg:
            for cb in range(NCB):
                for mt, (o0, osz) in enumerate(((0, 128), (128, 64))):
                    ps = psF.tile([osz, NCHUNK], F32, tag=f"o{mt}")
                    nc.tensor.matmul(
                        ps[:], wsb["wposxT"][:, o0:o0 + osz],
                        _canvas16_view(xc16[:], cb, 1, 1),
                        start=True, stop=False)
                    nc.tensor.matmul(
                        ps[:], wsb["wposyT"][:, o0:o0 + osz],
                        _canvas16_view(yc16[:], cb, 1, 1),
                        start=False, stop=False)
                    nc.tensor.matmul(
                        ps[:], m1T[:, o0:o0 + osz],
                        v_sb[:, cb * NCHUNK:(cb + 1) * NCHUNK],
                        start=False, stop=True)
                    osb = ostg.tile([osz, NCHUNK], F32, tag=f"os{mt}")
                    # halve PSUM-free latency: split every copy across both
                    # engines (keeps the PE out of mid-pstate micro-gaps)
                    h = NCHUNK // 2
                    nc.scalar.copy(out=osb[:, 0:h], in_=ps[:, 0:h])
                    nc.vector.tensor_copy(osb[:, h:], ps[:, h:])
                    oeng = nc.sync if (cb + mt) % 2 == 0 else nc.scalar
                    oeng.dma_start(
                        out=out_d[o0:o0 + osz,
                                  cb * NCHUNK:(cb + 1) * NCHUNK],
                        in_=osb[:])


_NC_CACHE = None


def kernel(x, y, w_pos, w_qv, w_qv_dw, w_kv, w_kv_dw, w_proj, w_fuse, b_fuse,
           temperature):
    global _NC_CACHE, LAST_RESULTS
    x = _f32(np.asarray(x))
    y = _f32(np.asarray(y))
    wts = _prep_weights(
        np.asarray(w_pos, np.float32), np.asarray(w_qv, np.float32),
        np.asarray(w_qv_dw, np.float32), np.asarray(w_kv, np.float32),
        np.asarray(w_kv_dw, np.float32), np.asarray(w_proj, np.float32),
        np.asarray(w_fuse, np.float32), np.asarray(b_fuse, np.float32),
        np.asarray(temperature, np.float32))

    if _NC_CACHE is None:
        _NC_CACHE = _build_nc()
    nc = _NC_CACHE

    in_maps = []
    for core in range(B):
        m = {"x": np.ascontiguousarray(x[core].reshape(C, N)),
             "y": np.ascontiguousarray(y[core].reshape(C, N))}
        m.update(wts)
        in_maps.append(m)

    res = run_bass_kernel_spmd(nc, in_maps, core_ids=list(range(B)),
                               trace=TRACE)
    LAST_RESULTS = res
    out = np.stack([np.asarray(r["out"]) for r in res.results])
    return out.reshape(B, 2 * C, H, W).astype(np.float32)


if __name__ == "__main__":
    print("built nc ok" if _build_nc() else "")
